# revision 29
# baseline (speedup 1.0000x reference)
"""Trainium2 Bass kernel for 3-layer GNN message passing with per-edge
multi-head attention over node history, distributed over 8 NeuronCores.

Sharding: nodes are relabeled by descending degree and dealt into
(superblock, core, slot) so that each 128-edge tile maps partition p <->
target slot p ("identity segment" scheme): the segment-sum one-hot matrix
becomes a constant identity, q is per-superblock constant (no per-edge q
gather), and tiles per superblock = max in-degree within the superblock
(near-optimal padding). Per-edge k/v history rows are assembled on the host
between launches (pure indexing) and streamed as dense bf16; v tables are
d-major permuted (via host weight-column permutation) so the attention-apply
multiply runs in the DVE 2x mode. All FLOPs run on device. 4 launches:
proj, layer1, layer2, layer3+head.
"""

import sys
import types

import numpy as np
import ml_dtypes

sys.path.insert(0, "/opt/trn_rl_repo")

BF16 = ml_dtypes.bfloat16

# ---------------------------------------------------------------- fixups
_HOOK = [None]


def _install_fixups():
    if "antenv.axon_hooks" not in sys.modules:
        mod = types.ModuleType("antenv.axon_hooks")
        mod.set_axon_ntff_profile_hook = lambda h: _HOOK.__setitem__(0, h)
        mod.get_axon_ntff_profile_hook = lambda: _HOOK[0]
        sys.modules["antenv.axon_hooks"] = mod
        try:
            from trn_agent_boot.trn_boot import _ntff_profile_via_ctypes

            _HOOK[0] = _ntff_profile_via_ctypes("/opt/axon/libaxon_pjrt.so")
        except Exception:
            pass

    import concourse.tile as tile
    from concourse.vector_clock import ScopedClock
    import bass_rust

    if getattr(tile.TileContext, "_drain_split_installed", False):
        return

    def _drain_and_barrier(self, tick_clock, wait_clock):
        nc = self.nc
        drain_inst = nc.sync.drain()
        wait_clock.add_sem_waits(
            drain_inst.ins, ScopedClock({None: tick_clock.global_clock})
        )
        si = drain_inst.ins.sync_info
        waits = list(si.on_wait or []) if si is not None else []
        if len(waits) > 1:
            si.on_wait = waits[:1]
            for i in range(1, len(waits)):
                d2 = nc.sync.drain()
                d2.ins.sync_info = bass_rust.SyncInfo(
                    on_wait=waits[i : i + 1], on_update=[]
                )
        nc.all_engine_barrier()
        assert self.sems is not None
        popped = nc._tile_sem_poison_stack.pop()
        assert popped is self._sem_poison
        nc.clear_and_free_semaphores(list(self.sems.allocated().values()))
        nc.all_engine_barrier()

    tile.TileContext._drain_and_barrier = _drain_and_barrier
    tile.TileContext._drain_split_installed = True


# ---------------------------------------------------------------- constants
N = 20000
E = 320000
IN_C = 256
HID = 64
OUT_C = 64
HEADS = 8
DH = 8
NCORES = 8
NPC = N // NCORES  # 2500
SBT = 128  # target slots per superblock
NSB = (NPC + SBT - 1) // SBT  # 20 (last has 68 targets)
LASTW = NPC - (NSB - 1) * SBT  # 68
MAXG = 40  # max tiles per streamed chunk

# d-major permutation of the 64 features (8 heads x 8 dims), an involution
PRM = np.arange(HID).reshape(HEADS, DH).T.reshape(-1)

_CACHE = {}


# ---------------------------------------------------------------- host prep
def _preprocess(edge_index):
    row = np.asarray(edge_index[0], dtype=np.int64)
    col = np.asarray(edge_index[1], dtype=np.int64)
    loop = np.arange(N, dtype=np.int64)
    row_all = np.concatenate([row, loop])
    col_all = np.concatenate([col, loop])
    deg = np.bincount(col_all, minlength=N).astype(np.int64)
    dinv = (1.0 / np.sqrt(np.maximum(deg, 1))).astype(np.float32)
    norm = (dinv[row_all] * dinv[col_all]).astype(np.float32)
    s_all = np.bincount(col_all, weights=norm.astype(np.float64), minlength=N)
    s_all = s_all.astype(np.float32)

    # degree-sorted relabeling: rank r -> (superblock b, core c, slot p)
    order = np.argsort(-deg, kind="stable")  # global ids by desc degree
    b_of = np.empty(N, np.int64)
    c_of = np.empty(N, np.int64)
    p_of = np.empty(N, np.int64)
    ranks = np.arange(N)
    full = (NSB - 1) * 1024  # ranks dealt in blocks of 8*128
    b_of[ranks < full] = ranks[ranks < full] // 1024
    c_of[ranks < full] = (ranks[ranks < full] % 1024) // SBT
    p_of[ranks < full] = ranks[ranks < full] % SBT
    tail = ranks >= full
    b_of[tail] = NSB - 1
    c_of[tail] = (ranks[tail] - full) // LASTW
    p_of[tail] = (ranks[tail] - full) % LASTW
    # per-node placement (indexed by global id)
    nb = np.empty(N, np.int64); nb[order] = b_of
    ncr = np.empty(N, np.int64); ncr[order] = c_of
    npp = np.empty(N, np.int64); npp[order] = p_of
    # ids[c][b*128+p] = global id owned by core c at local index
    ids = np.empty((NCORES, NPC), np.int64)
    loc = nb * SBT + npp  # local index within core
    ids[ncr, loc] = np.arange(N)

    # tiles per superblock = max degree within the superblock (desc sorted)
    tps = np.zeros(NSB, np.int64)
    for b in range(NSB):
        r0 = b * 1024 if b < NSB - 1 else full
        tps[b] = max(1, int(deg[order[r0]]))
    sb_start = np.zeros(NSB + 1, np.int64)
    sb_start[1:] = np.cumsum(tps)
    tt = int(sb_start[-1])

    # scatter edges: edge i (sorted by target) lands at
    # core c(t), row p(t), column sb_start[b(t)] + within-target-rank
    es = np.argsort(col_all, kind="stable")
    tgt = col_all[es]
    src = row_all[es]
    nm = norm[es]
    start_of = np.zeros(N + 1, np.int64)
    start_of[1:] = np.cumsum(np.bincount(tgt, minlength=N))
    rank_in_tgt = np.arange(len(tgt)) - start_of[tgt]
    dcol = sb_start[nb[tgt]] + rank_in_tgt
    drow = npp[tgt]
    dcore = ncr[tgt]

    metas = []
    for c in range(NCORES):
        m = dcore == c
        eidx = np.zeros((128, tt), np.int64)
        nrm = np.zeros((128, tt), np.float32)
        eidx[drow[m], dcol[m]] = src[m]
        nrm[drow[m], dcol[m]] = nm[m]
        metas.append(dict(eidx=eidx, nrm=nrm, nrmb=nrm.astype(BF16)))

    # chunk plan: per sb, tiles split into chunks of <= MAXG; sbs processed
    # smallest-first so the pipeline ramps quickly
    chunks = []  # (sb, t0, gw, first, last)
    for b in np.argsort(tps, kind="stable"):
        b = int(b)
        t0 = int(sb_start[b])
        left = int(tps[b])
        while left > 0:
            gw = min(MAXG, left)
            chunks.append(
                (b, t0, gw, t0 == int(sb_start[b]), left == gw)
            )
            t0 += gw
            left -= gw
    return metas, tuple(int(x) for x in tps), tt, chunks, s_all, ids


_WS_CTR = [0]


def _split_multi_waits(nc, maxw=1):
    """This container's walrus rejects instructions with more than one sync
    wait; hoist excess waits onto NoOps inserted before the instruction."""
    from concourse import mybir

    for f in nc.m.functions:
        for bb in f.blocks:
            insts = list(bb.instructions)
            out = []
            changed = False
            for inst in insts:
                si = inst.sync_info
                waits = list(si.on_wait) if (si is not None and si.on_wait) else []
                if len(waits) > maxw:
                    excess = waits[: len(waits) - maxw]
                    for j in range(0, len(excess), maxw):
                        _WS_CTR[0] += 1
                        out.append(
                            mybir.InstNoOp(
                                name=f"waitsplit_{_WS_CTR[0]}",
                                engine=inst.engine,
                                sync_info=mybir.SyncInfo(
                                    on_wait=excess[j : j + maxw], on_update=[]
                                ),
                                bass_nofuse=True,
                            )
                        )
                    si.on_wait = waits[len(waits) - maxw :]
                    changed = True
                out.append(inst)
            if changed:
                bb.instructions = out


def _mk_nc():
    import concourse.bass as bass

    return bass.Bass(num_devices=NCORES, debug=False, target_bir_lowering=False)


def _load_w(nc, pool, dram_ap, p, f, tag, dtype=None):
    from concourse import mybir

    t = pool.tile([p, f], dtype or mybir.dt.float32, tag=tag)
    nc.sync.dma_start(t[:], dram_ap[:])
    return t


def _proj_cols(nc, tc, ctx, w_t, b_t, srcs, out_slices, act_pool, psum_pool):
    """For each (src columnar tile [64, NPC], dram slice): write
    (w.T @ src + b) in bf16 to the dram slice, chunked by 500 cols."""
    from concourse import mybir

    f32 = mybir.dt.float32
    bf = mybir.dt.bfloat16
    Ident = mybir.ActivationFunctionType.Identity
    NCH = 500
    for (src, dst) in zip(srcs, out_slices):
        for j0 in range(0, NPC, NCH):
            w = min(NCH, NPC - j0)
            ps = psum_pool.tile([HID, NCH], f32, tag="proj")
            nc.tensor.matmul(
                out=ps[:, :w], lhsT=w_t[:], rhs=src[:, j0 : j0 + w],
                start=True, stop=True,
            )
            sb = act_pool.tile([HID, NCH], bf, tag="projsb")
            nc.scalar.activation(sb[:, :w], ps[:, :w], Ident, bias=b_t[:])
            nc.sync.dma_start(dst[:, j0 : j0 + w], sb[:, :w])


def _proj_cols_f32(nc, tc, ctx, w_t, b_t, src, dst, act_pool, psum_pool):
    """Single projection written as f32 (for q tables that the next launch
    transposes on device)."""
    from concourse import mybir

    f32 = mybir.dt.float32
    Ident = mybir.ActivationFunctionType.Identity
    NCH = 500
    for j0 in range(0, NPC, NCH):
        w = min(NCH, NPC - j0)
        ps = psum_pool.tile([HID, NCH], f32, tag="projq")
        nc.tensor.matmul(
            out=ps[:, :w], lhsT=w_t[:], rhs=src[:, j0 : j0 + w],
            start=True, stop=True,
        )
        sb = act_pool.tile([HID, NCH], f32, tag="projqsb")
        nc.scalar.activation(sb[:, :w], ps[:, :w], Ident, bias=b_t[:])
        nc.sync.dma_start(dst[:, j0 : j0 + w], sb[:, :w])


def _consts(nc, tc, ctx):
    from concourse import mybir
    from concourse.masks import make_identity

    cpool = ctx.enter_context(tc.tile_pool(name="const", bufs=1))
    ident_f = cpool.tile([128, 128], mybir.dt.float32, tag="idf")
    make_identity(nc, ident_f[:])
    ident_b = cpool.tile([128, 128], mybir.dt.bfloat16, tag="idb")
    nc.vector.tensor_copy(ident_b[:], ident_f[:])
    return ident_f, ident_b


def _qqt_from_cols(nc, tc, ctx, qT_d):
    """Load q column-table [64, NPC] f32 -> qqt [128, NSB, 128] bf16
    (rows [q; q] per superblock, zero-padded past NPC)."""
    from concourse import mybir

    f32 = mybir.dt.float32
    bf = mybir.dt.bfloat16
    Ident = mybir.ActivationFunctionType.Identity
    qpool = ctx.enter_context(tc.tile_pool(name="q", bufs=1))
    qT = qpool.tile([HID, NPC], f32, tag="qT")
    nc.sync.dma_start(qT[:], qT_d[:])
    qqt = qpool.tile([128, NSB, 128], bf, tag="qqt")
    nc.vector.memset(qqt[:], 0.0)
    for b in range(NSB):
        j0 = b * SBT
        w = min(SBT, NPC - j0)
        nc.scalar.activation(qqt[0:64, b, :w], qT[:, j0 : j0 + w], Ident)
        nc.scalar.activation(qqt[64:128, b, :w], qT[:, j0 : j0 + w], Ident)
    return qqt


def _qrows_from_cols(nc, tc, ctx, qT_d, ident_f):
    """Load q column-table [64, NPC] f32, transpose per superblock into
    qrows [128, NSB, 64] bf16 (row p = q of slot p; pad slots zeroed)."""
    from concourse import mybir

    f32 = mybir.dt.float32
    bf = mybir.dt.bfloat16
    qpool = ctx.enter_context(tc.tile_pool(name="q", bufs=1))
    qT = qpool.tile([HID, NPC], f32, tag="qT")
    nc.sync.dma_start(qT[:], qT_d[:])
    qrows = qpool.tile([128, NSB, HID], bf, tag="qrows")
    nc.vector.memset(qrows[:], 0.0)
    with tc.tile_pool(name="pqt", bufs=2, space="PSUM") as pst:
        for b in range(NSB):
            j0 = b * SBT
            w = min(SBT, NPC - j0)
            ps = pst.tile([128, HID], f32, tag="qtp")
            nc.tensor.transpose(
                out=ps[:w], in_=qT[:, j0 : j0 + w], identity=ident_f[:HID, :HID]
            )
            nc.scalar.copy(qrows[:w, b], ps[:w])
    return qrows


# ---------------------------------------------------------------- edge phase
def _edge_loop(nc, tc, ctx, chunks, streams, ident_b, compute_msg, out_cb,
               stageA=None, stageB=None):
    """Stream per-sb chunks; segment-sum via identity matmul (psT[64, 128] =
    sum_tiles msg.T). streams: list of (dram_ap, np, tag).

    Either compute_msg(tiles, b, t0, gw) -> msg, or a software-pipelined pair
    stageA(tiles, b, t0, gw) -> ctxobj (score side, ends on an ACT op) and
    stageB(ctxobj) -> msg: stageA of chunk i+1 is emitted before stageB of
    chunk i so the DVE works while ACT produces chunk i's activation."""
    from concourse import mybir

    f32 = mybir.dt.float32
    bf = mybir.dt.bfloat16
    ed_pool = ctx.enter_context(tc.tile_pool(name="ed", bufs=2))
    psum_seg = ctx.enter_context(tc.tile_pool(name="pseg", bufs=2, space="PSUM"))

    state = {"psT": None}

    def finish(b, t0, gw, first, last, aobj):
        msg = stageB(aobj) if stageB else aobj
        if first:
            psT_new = psum_seg.tile([HID, 128], f32, tag="psT")
            state["psT"] = psT_new
        psT = state["psT"]
        for gi in range(gw):
            nc.tensor.matmul(
                out=psT[:],
                lhsT=msg[:, gi],
                rhs=ident_b[:],
                start=(first and gi == 0),
                stop=(last and gi == gw - 1),
            )
        if last:
            out_cb(b, psT)

    pend = None
    for (b, t0, gw, first, last) in chunks:
        tiles = []
        for (ap, np_, tag) in streams:
            t = ed_pool.tile([np_, MAXG, ap.shape[2]], bf, tag=tag)
            nc.sync.dma_start(t[:, :gw], ap[:, t0 : t0 + gw, :])
            tiles.append(t)
        aobj = stageA(tiles, b, t0, gw) if stageA else compute_msg(tiles, b, t0, gw)
        if pend is not None:
            finish(*pend)
        pend = (b, t0, gw, first, last, aobj)
    if pend is not None:
        finish(*pend)


def _nrm_tiles(nc, tc, ctx, tt, nrm_d=None, nrmb_d=None):
    from concourse import mybir

    meta_pool = ctx.enter_context(tc.tile_pool(name="meta", bufs=1))
    nrm_t = None
    if nrm_d is not None:
        nrm_t = meta_pool.tile([128, tt], mybir.dt.float32, tag="nrmf")
        nc.sync.dma_start(nrm_t[:], nrm_d[:])
    nrmb_t = None
    if nrmb_d is not None:
        nrmb_t = meta_pool.tile([128, tt], mybir.dt.bfloat16, tag="nrmb")
        nc.sync.dma_start(nrmb_t[:], nrmb_d[:])
    return nrm_t, nrmb_t


# ---------------------------------------------------------------- launch A
def _build_launch_A():
    import concourse.tile as tile
    from concourse import mybir
    from contextlib import ExitStack

    f32 = mybir.dt.float32
    bf = mybir.dt.bfloat16
    nc = _mk_nc()
    xT = nc.dram_tensor("xT", [IN_C, NPC], bf, kind="ExternalInput").ap()
    w1 = nc.dram_tensor("w1", [IN_C, HID], bf, kind="ExternalInput").ap()
    b1 = nc.dram_tensor("b1", [HID, 1], f32, kind="ExternalInput").ap()
    hT_out = nc.dram_tensor("hT_out", [HID, NPC], bf, kind="ExternalOutput").ap()

    with tile.TileContext(nc) as tc, ExitStack() as ctx:
        wpool = ctx.enter_context(tc.tile_pool(name="w", bufs=1))
        xpool = ctx.enter_context(tc.tile_pool(name="x", bufs=1))
        hpool = ctx.enter_context(tc.tile_pool(name="h", bufs=1))
        psum_pool = ctx.enter_context(tc.tile_pool(name="ps", bufs=2, space="PSUM"))

        w1a = _load_w(nc, wpool, w1[0:128, :], 128, HID, "w1a", bf)
        w1b = _load_w(nc, wpool, w1[128:256, :], 128, HID, "w1b", bf)
        b1t = _load_w(nc, wpool, b1, HID, 1, "b1t")
        xa = xpool.tile([128, NPC], bf, tag="xa")
        xb = xpool.tile([128, NPC], bf, tag="xb")
        nc.sync.dma_start(xa[:], xT[0:128, :])
        nc.sync.dma_start(xb[:], xT[128:256, :])

        hT = hpool.tile([HID, NPC], bf)
        NCH = 500
        Relu = mybir.ActivationFunctionType.Relu
        for j0 in range(0, NPC, NCH):
            w = min(NCH, NPC - j0)
            ps = psum_pool.tile([HID, NCH], f32, tag="p1")
            nc.tensor.matmul(out=ps[:, :w], lhsT=w1a[:], rhs=xa[:, j0 : j0 + w], start=True, stop=False)
            nc.tensor.matmul(out=ps[:, :w], lhsT=w1b[:], rhs=xb[:, j0 : j0 + w], start=False, stop=True)
            nc.scalar.activation(hT[:, j0 : j0 + w], ps[:, :w], Relu, bias=b1t[:])
        nc.sync.dma_start(hT_out[:], hT[:])
    _split_multi_waits(nc)
    return nc


# ---------------------------------------------------------------- launch B (layer 1)
def _build_launch_B(tt, chunks):
    import concourse.tile as tile
    from concourse import mybir
    from contextlib import ExitStack

    f32 = mybir.dt.float32
    bf = mybir.dt.bfloat16
    AT = mybir.AluOpType
    Relu = mybir.ActivationFunctionType.Relu
    nc = _mk_nc()

    ed_d = nc.dram_tensor("ed", [128, tt, HID], bf, kind="ExternalInput").ap()
    nrm8b_d = nc.dram_tensor("nrm8b", [128, tt, 8], bf, kind="ExternalInput").ap()
    hT_d = nc.dram_tensor("hT", [HID, NPC], bf, kind="ExternalInput").ap()
    s8_d = nc.dram_tensor("s8", [8, NPC], bf, kind="ExternalInput").ap()
    bv08_d = nc.dram_tensor("bv08", [8, HID], bf, kind="ExternalInput").ap()
    wv0_d = nc.dram_tensor("wv0", [HID, HID], bf, kind="ExternalInput").ap()
    wk2_d = nc.dram_tensor("wk2", [HID, HID], bf, kind="ExternalInput").ap()
    wv2_d = nc.dram_tensor("wv2", [HID, HID], bf, kind="ExternalInput").ap()
    wq2_d = nc.dram_tensor("wq2", [HID, HID], bf, kind="ExternalInput").ap()
    bk2_d = nc.dram_tensor("bk2", [HID, 1], f32, kind="ExternalInput").ap()
    bv2_d = nc.dram_tensor("bv2", [HID, 1], f32, kind="ExternalInput").ap()
    bq2_d = nc.dram_tensor("bq2", [HID, 1], f32, kind="ExternalInput").ap()
    outT_d = nc.dram_tensor("outT", [HID, NPC], bf, kind="ExternalOutput").ap()
    cols_d = nc.dram_tensor("cols", [4 * HID, NPC], bf, kind="ExternalOutput").ap()
    q2T_d = nc.dram_tensor("q2T", [HID, NPC], f32, kind="ExternalOutput").ap()

    with tile.TileContext(nc) as tc, ExitStack() as ctx:
        ident_f, ident_b = _consts(nc, tc, ctx)
        meta_pool = ctx.enter_context(tc.tile_pool(name="meta", bufs=1))
        nrm8b_t = meta_pool.tile([128, tt, 8], bf, tag="nrm8b")
        nc.sync.dma_start(nrm8b_t[:], nrm8b_d[:])
        wpool = ctx.enter_context(tc.tile_pool(name="w", bufs=1))
        hpool = ctx.enter_context(tc.tile_pool(name="h", bufs=1))
        msg_pool = ctx.enter_context(tc.tile_pool(name="msg", bufs=2))
        act_pool = ctx.enter_context(tc.tile_pool(name="act", bufs=2))
        psum_o = ctx.enter_context(tc.tile_pool(name="po", bufs=2, space="PSUM"))
        psum_m = ctx.enter_context(tc.tile_pool(name="pm", bufs=2, space="PSUM"))

        wv0t = _load_w(nc, wpool, wv0_d, HID, HID, "wv0t", bf)
        wk2t = _load_w(nc, wpool, wk2_d, HID, HID, "wk2t", bf)
        wv2t = _load_w(nc, wpool, wv2_d, HID, HID, "wv2t", bf)
        wq2t = _load_w(nc, wpool, wq2_d, HID, HID, "wq2t", bf)
        bk2t = _load_w(nc, wpool, bk2_d, HID, 1, "bk2t")
        bv2t = _load_w(nc, wpool, bv2_d, HID, 1, "bv2t")
        bq2t = _load_w(nc, wpool, bq2_d, HID, 1, "bq2t")
        bv08t = _load_w(nc, wpool, bv08_d, 8, HID, "bv08t", bf)
        s8t = _load_w(nc, wpool, s8_d, 8, NPC, "s8t", bf)
        hT = hpool.tile([HID, NPC], bf, tag="hT")
        nc.sync.dma_start(hT[:], hT_d[:])
        outT = hpool.tile([HID, NPC], bf, tag="outT")

        def compute_msg(tiles, b, t0, gw):
            (ed_t,) = tiles
            msg = msg_pool.tile([128, MAXG, HID], bf, tag="msg")
            nc.vector.tensor_tensor(
                out=msg[:, :gw].rearrange("p c (a h) -> p c a h", h=8),
                in0=ed_t[:, :gw].rearrange("p c (a h) -> p c a h", h=8),
                in1=nrm8b_t[:, t0 : t0 + gw, None, :].to_broadcast([128, gw, 8, 8]),
                op=AT.mult,
            )
            return msg

        Ident = mybir.ActivationFunctionType.Identity

        def out_cb(b, psT):
            j0 = b * SBT
            w = min(SBT, NPC - j0)
            ST = act_pool.tile([HID, 128], bf, tag="ST")
            nc.scalar.copy(ST[:, :w], psT[:, :w])
            ps2 = psum_o.tile([HID, 128], f32, tag="ps2")
            nc.tensor.matmul(out=ps2[:, :w], lhsT=wv0t[:], rhs=ST[:, :w], start=True, stop=False)
            nc.tensor.matmul(out=ps2[:, :w], lhsT=bv08t[:], rhs=s8t[:, j0 : j0 + w], start=False, stop=True)
            nc.scalar.activation(outT[:, j0 : j0 + w], ps2[:, :w], Relu)
            # project this superblock's out1 columns immediately
            for (wt, bt, dst, odt) in (
                (wk2t, bk2t, cols_d[64:128, :], bf),
                (wv2t, bv2t, cols_d[192:256, :], bf),
                (wq2t, bq2t, q2T_d, f32),
            ):
                ps = psum_o.tile([HID, 128], f32, tag="ppb")
                nc.tensor.matmul(
                    out=ps[:, :w], lhsT=wt[:], rhs=outT[:, j0 : j0 + w],
                    start=True, stop=True,
                )
                sb_ = act_pool.tile([HID, 128], odt, tag=f"ppbs{odt}")
                nc.scalar.activation(sb_[:, :w], ps[:, :w], Ident, bias=bt[:])
                nc.sync.dma_start(dst[:, j0 : j0 + w], sb_[:, :w])

        # hT-sourced projections are independent of the edge loop; issue
        # them first so PE/ACT work on them while edge DMA ramps
        _proj_cols(nc, tc, ctx, wk2t, bk2t, [hT], [cols_d[0:64, :]], act_pool, psum_m)
        _proj_cols(nc, tc, ctx, wv2t, bv2t, [hT], [cols_d[128:192, :]], act_pool, psum_m)

        _edge_loop(nc, tc, ctx, chunks, [(ed_d, 128, "ed")], ident_b, compute_msg, out_cb)

        nc.sync.dma_start(outT_d[:], outT[:])
    _split_multi_waits(nc)
    return nc


# ---------------------------------------------------------------- launch C (layer 2)
def _build_launch_C(tt, chunks):
    import concourse.tile as tile
    from concourse import mybir
    from contextlib import ExitStack

    f32 = mybir.dt.float32
    bf = mybir.dt.bfloat16
    AT = mybir.AluOpType
    Relu = mybir.ActivationFunctionType.Relu
    Sig = mybir.ActivationFunctionType.Sigmoid
    nc = _mk_nc()
    roww = 4 * HID  # 256: [k0 k1 | v0 v1(d-major)]

    ed_d = nc.dram_tensor("ed", [128, tt, roww], bf, kind="ExternalInput").ap()
    nrm8b_d = nc.dram_tensor("nrm8b", [128, tt, 8], bf, kind="ExternalInput").ap()
    qT_d = nc.dram_tensor("qT", [HID, NPC], f32, kind="ExternalInput").ap()
    hT_d = nc.dram_tensor("hT", [HID, NPC], bf, kind="ExternalInput").ap()
    o1T_d = nc.dram_tensor("o1T", [HID, NPC], bf, kind="ExternalInput").ap()
    wk3a_d = nc.dram_tensor("wk3a", [HID, HID], bf, kind="ExternalInput").ap()
    wk3c_d = nc.dram_tensor("wk3c", [HID, HID], bf, kind="ExternalInput").ap()
    wv3a_d = nc.dram_tensor("wv3a", [HID, HID], bf, kind="ExternalInput").ap()
    wv3c_d = nc.dram_tensor("wv3c", [HID, HID], bf, kind="ExternalInput").ap()
    wq3_d = nc.dram_tensor("wq3", [HID, HID], bf, kind="ExternalInput").ap()
    bk3_d = nc.dram_tensor("bk3", [HID, 1], f32, kind="ExternalInput").ap()
    bv3_d = nc.dram_tensor("bv3", [HID, 1], f32, kind="ExternalInput").ap()
    bq3_d = nc.dram_tensor("bq3", [HID, 1], f32, kind="ExternalInput").ap()
    cols_d = nc.dram_tensor("cols", [6 * HID, NPC], bf, kind="ExternalOutput").ap()
    q3T_d = nc.dram_tensor("q3T", [HID, NPC], f32, kind="ExternalOutput").ap()

    with tile.TileContext(nc) as tc, ExitStack() as ctx:
        ident_f, ident_b = _consts(nc, tc, ctx)
        meta_pool = ctx.enter_context(tc.tile_pool(name="meta", bufs=1))
        nrm8b_t = meta_pool.tile([128, tt, 8], bf, tag="nrm8b")
        nc.sync.dma_start(nrm8b_t[:], nrm8b_d[:])
        qrows = _qrows_from_cols(nc, tc, ctx, qT_d, ident_f)
        wpool = ctx.enter_context(tc.tile_pool(name="w", bufs=1))
        hpool = ctx.enter_context(tc.tile_pool(name="h", bufs=1))
        dk_pool = ctx.enter_context(tc.tile_pool(name="dk", bufs=2))
        sc_pool = ctx.enter_context(tc.tile_pool(name="sc", bufs=2))
        msg_pool = ctx.enter_context(tc.tile_pool(name="msg", bufs=2))
        act_pool = ctx.enter_context(tc.tile_pool(name="act", bufs=2))
        psum_m = ctx.enter_context(tc.tile_pool(name="pm", bufs=2, space="PSUM"))

        wk3at = _load_w(nc, wpool, wk3a_d, HID, HID, "wk3at", bf)
        wk3ct = _load_w(nc, wpool, wk3c_d, HID, HID, "wk3ct", bf)
        wv3at = _load_w(nc, wpool, wv3a_d, HID, HID, "wv3at", bf)
        wv3ct = _load_w(nc, wpool, wv3c_d, HID, HID, "wv3ct", bf)
        wq3t = _load_w(nc, wpool, wq3_d, HID, HID, "wq3t", bf)
        bk3t = _load_w(nc, wpool, bk3_d, HID, 1, "bk3t")
        bv3t = _load_w(nc, wpool, bv3_d, HID, 1, "bv3t")
        bq3t = _load_w(nc, wpool, bq3_d, HID, 1, "bq3t")
        hT = hpool.tile([HID, NPC], bf, tag="hT")
        nc.sync.dma_start(hT[:], hT_d[:])
        o1T = hpool.tile([HID, NPC], bf, tag="o1T")
        nc.sync.dma_start(o1T[:], o1T_d[:])
        o2T = hpool.tile([HID, NPC], bf, tag="o2T")

        def stageA(tiles, b, t0, gw):
            (ed_t,) = tiles
            ke = ed_t[:, :gw, 0 : 2 * HID].rearrange("p c (t d) -> p c t d", t=2)
            dk = dk_pool.tile([128, MAXG, 2, HID], bf, tag="dk")
            nc.vector.tensor_tensor(
                out=dk[:, :gw], in0=ke,
                in1=qrows[:, b : b + 1, None, :].to_broadcast([128, gw, 2, HID]),
                op=AT.mult,
            )
            dk5 = dk[:, :gw].rearrange("p c t (h d) -> p c t h d", h=8)
            r4 = sc_pool.tile([128, MAXG, 2, 8, 4], bf, tag="r4")
            nc.vector.tensor_tensor(
                out=r4[:, :gw], in0=dk5[:, :, :, :, 0:4], in1=dk5[:, :, :, :, 4:8], op=AT.add
            )
            r2 = sc_pool.tile([128, MAXG, 2, 8, 2], bf, tag="r2")
            nc.vector.tensor_tensor(
                out=r2[:, :gw], in0=r4[:, :gw, :, :, 0:2], in1=r4[:, :gw, :, :, 2:4], op=AT.add
            )
            sc = sc_pool.tile([128, MAXG, 2, 8], f32, tag="sc")
            nc.vector.tensor_tensor(
                out=sc[:, :gw], in0=r2[:, :gw, :, :, 0], in1=r2[:, :gw, :, :, 1], op=AT.add
            )
            z = sc_pool.tile([128, MAXG, 8], f32, tag="z")
            nc.vector.tensor_tensor(out=z[:, :gw], in0=sc[:, :gw, 0], in1=sc[:, :gw, 1], op=AT.subtract)
            a0 = sc_pool.tile([128, MAXG, 8], bf, tag="a0")
            nc.scalar.activation(a0[:, :gw], z[:, :gw], Sig)
            return (ed_t, a0, t0, gw)

        def stageB(aobj):
            (ed_t, a0, t0, gw) = aobj
            an0 = sc_pool.tile([128, MAXG, 8], bf, tag="an0")
            nc.vector.tensor_tensor(
                out=an0[:, :gw], in0=a0[:, :gw],
                in1=nrm8b_t[:, t0 : t0 + gw], op=AT.mult,
            )
            an1 = sc_pool.tile([128, MAXG, 8], bf, tag="an1")
            nc.vector.tensor_tensor(
                out=an1[:, :gw], in0=nrm8b_t[:, t0 : t0 + gw], in1=an0[:, :gw],
                op=AT.subtract,
            )
            ve = ed_t[:, :gw, 2 * HID : 4 * HID].rearrange(
                "p c (t d h) -> p c t d h", t=2, d=8
            )
            wv0_ = msg_pool.tile([128, MAXG, 8, 8], bf, tag="wv0")
            nc.vector.tensor_tensor(
                out=wv0_[:, :gw], in0=ve[:, :, 0],
                in1=an0[:, :gw, None, :].to_broadcast([128, gw, 8, 8]),
                op=AT.mult,
            )
            wv1_ = msg_pool.tile([128, MAXG, 8, 8], bf, tag="wv1")
            nc.vector.tensor_tensor(
                out=wv1_[:, :gw], in0=ve[:, :, 1],
                in1=an1[:, :gw, None, :].to_broadcast([128, gw, 8, 8]),
                op=AT.mult,
            )
            msg = msg_pool.tile([128, MAXG, HID], bf, tag="msg")
            nc.vector.tensor_tensor(
                out=msg[:, :gw].rearrange("p c (d h) -> p c d h", d=8),
                in0=wv0_[:, :gw], in1=wv1_[:, :gw], op=AT.add,
            )
            return msg

        Ident = mybir.ActivationFunctionType.Identity
        psum_p = ctx.enter_context(tc.tile_pool(name="pp", bufs=2, space="PSUM"))

        def out_cb(b, psT):
            j0 = b * SBT
            w = min(SBT, NPC - j0)
            nc.scalar.activation(o2T[:, j0 : j0 + w], psT[:, :w], Relu)
            # project this superblock's columns immediately (overlaps edge loop)
            for (wt, bt, dst) in (
                (wk3ct, bk3t, cols_d[128:192, :]),
                (wv3ct, bv3t, cols_d[320:384, :]),
            ):
                ps = psum_p.tile([HID, 128], f32, tag="pp")
                nc.tensor.matmul(
                    out=ps[:, :w], lhsT=wt[:], rhs=o2T[:, j0 : j0 + w],
                    start=True, stop=True,
                )
                sb_ = act_pool.tile([HID, 128], bf, tag="ppsb")
                nc.scalar.activation(sb_[:, :w], ps[:, :w], Ident, bias=bt[:])
                nc.sync.dma_start(dst[:, j0 : j0 + w], sb_[:, :w])
            ps = psum_p.tile([HID, 128], f32, tag="pp")
            nc.tensor.matmul(
                out=ps[:, :w], lhsT=wq3t[:], rhs=o2T[:, j0 : j0 + w],
                start=True, stop=True,
            )
            sbq = act_pool.tile([HID, 128], f32, tag="ppsbq")
            nc.scalar.activation(sbq[:, :w], ps[:, :w], Ident, bias=bq3t[:])
            nc.sync.dma_start(q3T_d[:, j0 : j0 + w], sbq[:, :w])

        _proj_cols(
            nc, tc, ctx, wk3at, bk3t, [hT, o1T],
            [cols_d[0:64, :], cols_d[64:128, :]], act_pool, psum_m,
        )
        _proj_cols(
            nc, tc, ctx, wv3at, bv3t, [hT, o1T],
            [cols_d[192:256, :], cols_d[256:320, :]], act_pool, psum_m,
        )

        _edge_loop(
            nc, tc, ctx, chunks, [(ed_d, 128, "ed")], ident_b, None, out_cb,
            stageA=stageA, stageB=stageB,
        )
    _split_multi_waits(nc)
    return nc


# ---------------------------------------------------------------- launch D (layer 3 + head)
def _build_launch_D(tt, chunks):
    import concourse.tile as tile
    from concourse import mybir
    from contextlib import ExitStack

    f32 = mybir.dt.float32
    bf = mybir.dt.bfloat16
    AT = mybir.AluOpType
    Relu = mybir.ActivationFunctionType.Relu
    Exp = mybir.ActivationFunctionType.Exp
    Ln = mybir.ActivationFunctionType.Ln
    nc = _mk_nc()
    roww = 6 * HID  # 384: [k0 k1 k2 | v0 v1 v2(d-major)]

    ed_d = nc.dram_tensor("ed", [128, tt, roww], bf, kind="ExternalInput").ap()
    nrm8b_d = nc.dram_tensor("nrm8b", [128, tt, 8], bf, kind="ExternalInput").ap()
    qT_d = nc.dram_tensor("qT", [HID, NPC], f32, kind="ExternalInput").ap()
    w2_d = nc.dram_tensor("w2", [HID, OUT_C], bf, kind="ExternalInput").ap()
    b2bc_d = nc.dram_tensor("b2bc", [128, OUT_C], f32, kind="ExternalInput").ap()
    y_d = nc.dram_tensor("y", [NPC, OUT_C], f32, kind="ExternalOutput").ap()

    with tile.TileContext(nc) as tc, ExitStack() as ctx:
        ident_f, ident_b = _consts(nc, tc, ctx)
        meta_pool = ctx.enter_context(tc.tile_pool(name="meta", bufs=1))
        nrm8b_t = meta_pool.tile([128, tt, 8], bf, tag="nrm8b")
        nc.sync.dma_start(nrm8b_t[:], nrm8b_d[:])
        qrows = _qrows_from_cols(nc, tc, ctx, qT_d, ident_f)
        wpool = ctx.enter_context(tc.tile_pool(name="w", bufs=1))
        dk_pool = ctx.enter_context(tc.tile_pool(name="dk", bufs=2))
        sc_pool = ctx.enter_context(tc.tile_pool(name="sc", bufs=2))
        msg_pool = ctx.enter_context(tc.tile_pool(name="msg", bufs=2))
        act_pool = ctx.enter_context(tc.tile_pool(name="act", bufs=2))
        sm_pool = ctx.enter_context(tc.tile_pool(name="sm", bufs=2))
        psum_lg = ctx.enter_context(tc.tile_pool(name="plg", bufs=2, space="PSUM"))

        w2t = _load_w(nc, wpool, w2_d, HID, OUT_C, "w2t", bf)
        b2t = _load_w(nc, wpool, b2bc_d, 128, OUT_C, "b2t")

        def stageA(tiles, b, t0, gw):
            (ed_t,) = tiles
            ke = ed_t[:, :gw, 0 : 3 * HID].rearrange("p c (t d) -> p c t d", t=3)
            dk = dk_pool.tile([128, MAXG, 3, HID], bf, tag="dk")
            nc.vector.tensor_tensor(
                out=dk[:, :gw], in0=ke,
                in1=qrows[:, b : b + 1, None, :].to_broadcast([128, gw, 3, HID]),
                op=AT.mult,
            )
            dk5 = dk[:, :gw].rearrange("p c t (h d) -> p c t h d", h=8)
            r4 = sc_pool.tile([128, MAXG, 3, 8, 4], bf, tag="r4")
            nc.vector.tensor_tensor(
                out=r4[:, :gw], in0=dk5[:, :, :, :, 0:4], in1=dk5[:, :, :, :, 4:8], op=AT.add
            )
            r2 = sc_pool.tile([128, MAXG, 3, 8, 2], bf, tag="r2")
            nc.vector.tensor_tensor(
                out=r2[:, :gw], in0=r4[:, :gw, :, :, 0:2], in1=r4[:, :gw, :, :, 2:4], op=AT.add
            )
            sc = sc_pool.tile([128, MAXG, 3, 8], f32, tag="sc")
            nc.vector.tensor_tensor(
                out=sc[:, :gw], in0=r2[:, :gw, :, :, 0], in1=r2[:, :gw, :, :, 1], op=AT.add
            )
            ee = sc_pool.tile([128, MAXG, 3, 8], bf, tag="ee")
            nc.scalar.activation(ee[:, :gw], sc[:, :gw], Exp)
            return (ed_t, ee, t0, gw)

        def stageB(aobj):
            (ed_t, ee, t0, gw) = aobj
            dd1 = sc_pool.tile([128, MAXG, 8], bf, tag="dd1")
            nc.vector.tensor_tensor(out=dd1[:, :gw], in0=ee[:, :gw, 0], in1=ee[:, :gw, 1], op=AT.add)
            dd = sc_pool.tile([128, MAXG, 8], f32, tag="dd")
            nc.vector.tensor_tensor(out=dd[:, :gw], in0=dd1[:, :gw], in1=ee[:, :gw, 2], op=AT.add)
            rr = sc_pool.tile([128, MAXG, 8], bf, tag="rr")
            with nc.allow_low_precision(reason="attn denom fine in bf16"):
                nc.vector.reciprocal(rr[:, :gw], dd[:, :gw])
            rn = sc_pool.tile([128, MAXG, 8], bf, tag="rn")
            nc.vector.tensor_tensor(
                out=rn[:, :gw], in0=rr[:, :gw],
                in1=nrm8b_t[:, t0 : t0 + gw], op=AT.mult,
            )
            aa = sc_pool.tile([128, MAXG, 3, 8], bf, tag="aa")
            nc.vector.tensor_tensor(
                out=aa[:, :gw], in0=ee[:, :gw],
                in1=rn[:, :gw, None, :].to_broadcast([128, gw, 3, 8]),
                op=AT.mult,
            )
            ve = ed_t[:, :gw, 3 * HID : 6 * HID].rearrange(
                "p c (t d h) -> p c t d h", t=3, d=8
            )
            wv_ = msg_pool.tile([128, MAXG, 3, 8, 8], bf, tag="wv")
            nc.vector.tensor_tensor(
                out=wv_[:, :gw], in0=ve,
                in1=aa[:, :gw, :, None, :].to_broadcast([128, gw, 3, 8, 8]),
                op=AT.mult,
            )
            msg1 = msg_pool.tile([128, MAXG, 8, 8], bf, tag="msg1")
            nc.vector.tensor_tensor(out=msg1[:, :gw], in0=wv_[:, :gw, 0], in1=wv_[:, :gw, 1], op=AT.add)
            msg = msg_pool.tile([128, MAXG, HID], bf, tag="msg")
            nc.vector.tensor_tensor(
                out=msg[:, :gw].rearrange("p c (d h) -> p c d h", d=8),
                in0=msg1[:, :gw], in1=wv_[:, :gw, 2], op=AT.add,
            )
            return msg

        def out_cb(b, psT):
            j0 = b * SBT
            w = min(SBT, NPC - j0)
            o3T = act_pool.tile([HID, 128], bf, tag="o3T")
            nc.scalar.activation(o3T[:, :w], psT[:, :w], Relu)
            lg = psum_lg.tile([128, OUT_C], f32, tag="lg")
            nc.tensor.matmul(out=lg[:w], lhsT=o3T[:, :w], rhs=w2t[:], start=True, stop=True)
            logits = sm_pool.tile([128, OUT_C], f32, tag="logits")
            nc.vector.tensor_tensor(out=logits[:w], in0=lg[:w], in1=b2t[:w], op=AT.add)
            nlmax = sm_pool.tile([128, 1], f32, tag="nlmax")
            nc.vector.tensor_reduce(
                out=nlmax[:w], in_=logits[:w], axis=mybir.AxisListType.X,
                op=AT.max, negate=True,
            )
            eb = sm_pool.tile([128, OUT_C], f32, tag="eb")
            esum = sm_pool.tile([128, 1], f32, tag="esum")
            nc.scalar.activation(
                eb[:w], logits[:w], Exp, bias=nlmax[:w], accum_out=esum[:w]
            )
            lse = sm_pool.tile([128, 1], f32, tag="lse")
            nc.scalar.activation(lse[:w], esum[:w], Ln)
            off = sm_pool.tile([128, 1], f32, tag="off")
            nc.vector.tensor_tensor(out=off[:w], in0=lse[:w], in1=nlmax[:w], op=AT.subtract)
            yy = sm_pool.tile([128, OUT_C], f32, tag="yy")
            nc.vector.tensor_tensor(
                out=yy[:w], in0=logits[:w],
                in1=off[:w].to_broadcast([w, OUT_C]), op=AT.subtract,
            )
            nc.sync.dma_start(y_d[j0 : j0 + w, :], yy[:w])

        _edge_loop(
            nc, tc, ctx, chunks, [(ed_d, 128, "ed")], ident_b, None, out_cb,
            stageA=stageA, stageB=stageB,
        )
    _split_multi_waits(nc)
    return nc


# ---------------------------------------------------------------- host gather
def _u16(a):
    return a.view(np.uint16)


def _gather_rows(tab, eidx):
    """[128, TT, W] bf16 rows gathered by global src id."""
    return _u16(tab)[eidx].view(BF16)


def _gather_ed(ktab, vtab, eidx):
    """[128, TT, kw+vw] bf16: k rows then v rows (both by global src id)."""
    tt = eidx.shape[1]
    kw = ktab.shape[1]
    vw = vtab.shape[1]
    out = np.empty((128, tt, kw + vw), dtype=np.uint16)
    out[:, :, :kw] = _u16(ktab)[eidx]
    out[:, :, kw:] = _u16(vtab)[eidx]
    return out.view(BF16)


def _nrm8(meta):
    n8 = np.ascontiguousarray(
        np.broadcast_to(meta["nrm"][:, :, None], meta["nrm"].shape + (8,))
    )
    return n8, n8.astype(BF16)


def _scatter_tab(cols_list, ids, lo, hi, dtype=BF16):
    """tab[global_id] = cols[lo:hi].T for each core."""
    tab = np.empty((N, hi - lo), dtype=dtype)
    for c in range(NCORES):
        tab[ids[c]] = cols_list[c][lo:hi].T
    return tab


# ---------------------------------------------------------------- driver
def kernel(x, edge_index, lin1_w, lin1_b, wq, bq, wk, bk, wv, bv, lin2_w, lin2_b):
    _install_fixups()
    from concourse.bass_utils import run_bass_kernel_spmd

    x = np.asarray(x, dtype=np.float32)
    lin1_w = np.asarray(lin1_w, np.float32)
    lin1_b = np.asarray(lin1_b, np.float32)
    wq = np.asarray(wq, np.float32)
    bq = np.asarray(bq, np.float32)
    wk = np.asarray(wk, np.float32)
    bk = np.asarray(bk, np.float32)
    wv = np.asarray(wv, np.float32)
    bv = np.asarray(bv, np.float32)
    lin2_w = np.asarray(lin2_w, np.float32)
    lin2_b = np.asarray(lin2_b, np.float32)
    isd = np.float32(1.0 / np.sqrt(DH))

    metas, tps, tt, chunks, s_all, ids = _preprocess(np.asarray(edge_index))

    key = ("progs", tps, tt)
    if key not in _CACHE:
        _CACHE[key] = (
            _build_launch_A(),
            _build_launch_B(tt, chunks),
            _build_launch_C(tt, chunks),
            _build_launch_D(tt, chunks),
        )
    ncA, ncB, ncC, ncD = _CACHE[key]
    cores = list(range(NCORES))

    # ---- launch A: h = relu(x @ W1 + b1), columnar bf16
    xT = np.ascontiguousarray(x.T).astype(BF16)
    w1_bf = lin1_w.astype(BF16)
    in_maps = [
        dict(
            xT=np.ascontiguousarray(xT[:, ids[c]]),
            w1=w1_bf,
            b1=lin1_b[:, None],
        )
        for c in cores
    ]
    resA = run_bass_kernel_spmd(ncA, in_maps, cores)
    hT = [np.asarray(resA.results[c]["hT_out"]) for c in cores]
    h_tab = np.empty((N, HID), dtype=BF16)
    for c in cores:
        h_tab[ids[c]] = hT[c].T

    # ---- launch B: layer 1 (attn == identity) + k2/v2/q2 tables
    nrm8b_all = []
    for c in cores:
        _, bb = _nrm8(metas[c])
        nrm8b_all.append(bb)
    s8 = []
    for c in cores:
        a = np.zeros((8, NPC), dtype=BF16)
        a[0] = s_all[ids[c]].astype(BF16)
        s8.append(a)
    bv08 = np.zeros((8, HID), dtype=BF16)
    bv08[0] = bv[0].astype(BF16)
    in_maps = [
        dict(
            ed=_gather_rows(h_tab, metas[c]["eidx"]),
            nrm8b=nrm8b_all[c],
            hT=hT[c],
            s8=s8[c],
            bv08=bv08,
            wv0=wv[0].astype(BF16),
            wk2=wk[1].astype(BF16),
            wv2=wv[1][:, PRM].astype(BF16),
            wq2=(wq[1] * isd).astype(BF16),
            bk2=bk[1][:, None],
            bv2=bv[1][PRM][:, None],
            bq2=(bq[1] * isd)[:, None],
        )
        for c in cores
    ]
    resB = run_bass_kernel_spmd(ncB, in_maps, cores)
    o1T = [np.asarray(resB.results[c]["outT"]) for c in cores]
    colsB = [np.asarray(resB.results[c]["cols"]) for c in cores]
    q2T = [np.asarray(resB.results[c]["q2T"]) for c in cores]
    ktab2 = _scatter_tab(colsB, ids, 0, 128)
    vtab2 = _scatter_tab(colsB, ids, 128, 256)

    # ---- launch C: layer 2 + k3/v3/q3 tables
    in_maps = [
        dict(
            ed=_gather_ed(ktab2, vtab2, metas[c]["eidx"]),
            nrm8b=nrm8b_all[c],
            qT=q2T[c],
            hT=hT[c],
            o1T=o1T[c],
            wk3a=wk[2].astype(BF16),
            wk3c=wk[2][PRM, :].astype(BF16),
            wv3a=wv[2][:, PRM].astype(BF16),
            wv3c=wv[2][PRM, :][:, PRM].astype(BF16),
            wq3=((wq[2] * isd)[PRM, :]).astype(BF16),
            bk3=bk[2][:, None],
            bv3=bv[2][PRM][:, None],
            bq3=(bq[2] * isd)[:, None],
        )
        for c in cores
    ]
    resC = run_bass_kernel_spmd(ncC, in_maps, cores)
    colsC = [np.asarray(resC.results[c]["cols"]) for c in cores]
    q3T = [np.asarray(resC.results[c]["q3T"]) for c in cores]
    ktab3 = _scatter_tab(colsC, ids, 0, 192)
    vtab3 = _scatter_tab(colsC, ids, 192, 384)

    # ---- launch D: layer 3 + classifier head + log_softmax
    b2bc = np.ascontiguousarray(np.broadcast_to(lin2_b[None, :], (128, OUT_C)))
    in_maps = [
        dict(
            ed=_gather_ed(ktab3, vtab3, metas[c]["eidx"]),
            nrm8b=nrm8b_all[c],
            qT=q3T[c],
            w2=lin2_w[PRM, :].astype(BF16),
            b2bc=b2bc,
        )
        for c in cores
    ]
    resD = run_bass_kernel_spmd(ncD, in_maps, cores)
    y = np.empty((N, OUT_C), dtype=np.float32)
    for c in cores:
        y[ids[c]] = np.asarray(resD.results[c]["y"], dtype=np.float32)
    return y


# revision 30
# speedup vs baseline: 1.2388x; 1.2388x over previous
"""Trainium2 Bass kernel for 3-layer GNN message passing with per-edge
multi-head attention over node history, distributed over 8 NeuronCores.

Sharding: nodes are relabeled by descending degree and dealt into
(superblock, core, slot) so that each 128-edge tile maps partition p <->
target slot p ("identity segment" scheme): the segment-sum one-hot matrix
becomes a constant identity, q is per-superblock constant (no per-edge q
gather), and tiles per superblock = max in-degree within the superblock
(near-optimal padding). Per-edge k/v history rows are assembled on the host
between launches (pure indexing) and streamed as dense bf16; v tables are
d-major permuted (via host weight-column permutation) so the attention-apply
multiply runs in the DVE 2x mode. All FLOPs run on device. 4 launches:
proj, layer1, layer2, layer3+head.
"""

import sys
import types

import numpy as np
import ml_dtypes

sys.path.insert(0, "/opt/trn_rl_repo")

BF16 = ml_dtypes.bfloat16

# ---------------------------------------------------------------- fixups
_HOOK = [None]


def _install_fixups():
    if "antenv.axon_hooks" not in sys.modules:
        mod = types.ModuleType("antenv.axon_hooks")
        mod.set_axon_ntff_profile_hook = lambda h: _HOOK.__setitem__(0, h)
        mod.get_axon_ntff_profile_hook = lambda: _HOOK[0]
        sys.modules["antenv.axon_hooks"] = mod
        try:
            from trn_agent_boot.trn_boot import _ntff_profile_via_ctypes

            _HOOK[0] = _ntff_profile_via_ctypes("/opt/axon/libaxon_pjrt.so")
        except Exception:
            pass

    import concourse.tile as tile
    from concourse.vector_clock import ScopedClock
    import bass_rust

    if getattr(tile.TileContext, "_drain_split_installed", False):
        return

    def _drain_and_barrier(self, tick_clock, wait_clock):
        nc = self.nc
        drain_inst = nc.sync.drain()
        wait_clock.add_sem_waits(
            drain_inst.ins, ScopedClock({None: tick_clock.global_clock})
        )
        si = drain_inst.ins.sync_info
        waits = list(si.on_wait or []) if si is not None else []
        if len(waits) > 1:
            si.on_wait = waits[:1]
            for i in range(1, len(waits)):
                d2 = nc.sync.drain()
                d2.ins.sync_info = bass_rust.SyncInfo(
                    on_wait=waits[i : i + 1], on_update=[]
                )
        nc.all_engine_barrier()
        assert self.sems is not None
        popped = nc._tile_sem_poison_stack.pop()
        assert popped is self._sem_poison
        nc.clear_and_free_semaphores(list(self.sems.allocated().values()))
        nc.all_engine_barrier()

    tile.TileContext._drain_and_barrier = _drain_and_barrier
    tile.TileContext._drain_split_installed = True


# ---------------------------------------------------------------- constants
N = 20000
E = 320000
IN_C = 256
HID = 64
OUT_C = 64
HEADS = 8
DH = 8
NCORES = 8
NPC = N // NCORES  # 2500
SBT = 128  # target slots per superblock
NSB = (NPC + SBT - 1) // SBT  # 20 (last has 68 targets)
LASTW = NPC - (NSB - 1) * SBT  # 68
MAXG = 32  # max tiles per streamed chunk

# d-major permutation of the 64 features (8 heads x 8 dims), an involution
PRM = np.arange(HID).reshape(HEADS, DH).T.reshape(-1)

_CACHE = {}


# ---------------------------------------------------------------- host prep
def _preprocess(edge_index):
    row = np.asarray(edge_index[0], dtype=np.int64)
    col = np.asarray(edge_index[1], dtype=np.int64)
    loop = np.arange(N, dtype=np.int64)
    row_all = np.concatenate([row, loop])
    col_all = np.concatenate([col, loop])
    deg = np.bincount(col_all, minlength=N).astype(np.int64)
    dinv = (1.0 / np.sqrt(np.maximum(deg, 1))).astype(np.float32)
    norm = (dinv[row_all] * dinv[col_all]).astype(np.float32)
    s_all = np.bincount(col_all, weights=norm.astype(np.float64), minlength=N)
    s_all = s_all.astype(np.float32)

    # degree-sorted relabeling: rank r -> (superblock b, core c, slot p)
    order = np.argsort(-deg, kind="stable")  # global ids by desc degree
    b_of = np.empty(N, np.int64)
    c_of = np.empty(N, np.int64)
    p_of = np.empty(N, np.int64)
    ranks = np.arange(N)
    full = (NSB - 1) * 1024  # ranks dealt in blocks of 8*128
    b_of[ranks < full] = ranks[ranks < full] // 1024
    c_of[ranks < full] = (ranks[ranks < full] % 1024) // SBT
    p_of[ranks < full] = ranks[ranks < full] % SBT
    tail = ranks >= full
    b_of[tail] = NSB - 1
    c_of[tail] = (ranks[tail] - full) // LASTW
    p_of[tail] = (ranks[tail] - full) % LASTW
    # per-node placement (indexed by global id)
    nb = np.empty(N, np.int64); nb[order] = b_of
    ncr = np.empty(N, np.int64); ncr[order] = c_of
    npp = np.empty(N, np.int64); npp[order] = p_of
    # ids[c][b*128+p] = global id owned by core c at local index
    ids = np.empty((NCORES, NPC), np.int64)
    loc = nb * SBT + npp  # local index within core
    ids[ncr, loc] = np.arange(N)

    # tiles per superblock = max degree within the superblock (desc sorted)
    tps = np.zeros(NSB, np.int64)
    for b in range(NSB):
        r0 = b * 1024 if b < NSB - 1 else full
        tps[b] = max(1, int(deg[order[r0]]))
    sb_start = np.zeros(NSB + 1, np.int64)
    sb_start[1:] = np.cumsum(tps)
    tt = int(sb_start[-1])

    # scatter edges: edge i (sorted by target) lands at
    # core c(t), row p(t), column sb_start[b(t)] + within-target-rank
    es = np.argsort(col_all, kind="stable")
    tgt = col_all[es]
    src = row_all[es]
    nm = norm[es]
    start_of = np.zeros(N + 1, np.int64)
    start_of[1:] = np.cumsum(np.bincount(tgt, minlength=N))
    rank_in_tgt = np.arange(len(tgt)) - start_of[tgt]
    dcol = sb_start[nb[tgt]] + rank_in_tgt
    drow = npp[tgt]
    dcore = ncr[tgt]

    metas = []
    for c in range(NCORES):
        m = dcore == c
        eidx = np.zeros((128, tt), np.int64)
        nrm = np.zeros((128, tt), np.float32)
        eidx[drow[m], dcol[m]] = src[m]
        nrm[drow[m], dcol[m]] = nm[m]
        metas.append(dict(eidx=eidx, nrm=nrm, nrmb=nrm.astype(BF16)))

    # chunk plan: per sb, tiles split into chunks of <= MAXG; sbs processed
    # smallest-first so the pipeline ramps quickly
    chunks = []  # (sb, t0, gw, first, last)
    for b in np.argsort(tps, kind="stable"):
        b = int(b)
        t0 = int(sb_start[b])
        left = int(tps[b])
        while left > 0:
            gw = min(MAXG, left)
            chunks.append(
                (b, t0, gw, t0 == int(sb_start[b]), left == gw)
            )
            t0 += gw
            left -= gw
    return metas, tuple(int(x) for x in tps), tt, chunks, s_all, ids


_WS_CTR = [0]


def _split_multi_waits(nc, maxw=1):
    """This container's walrus rejects instructions with more than one sync
    wait; hoist excess waits onto NoOps inserted before the instruction."""
    from concourse import mybir

    for f in nc.m.functions:
        for bb in f.blocks:
            insts = list(bb.instructions)
            out = []
            changed = False
            for inst in insts:
                si = inst.sync_info
                waits = list(si.on_wait) if (si is not None and si.on_wait) else []
                if len(waits) > maxw:
                    excess = waits[: len(waits) - maxw]
                    for j in range(0, len(excess), maxw):
                        _WS_CTR[0] += 1
                        out.append(
                            mybir.InstNoOp(
                                name=f"waitsplit_{_WS_CTR[0]}",
                                engine=inst.engine,
                                sync_info=mybir.SyncInfo(
                                    on_wait=excess[j : j + maxw], on_update=[]
                                ),
                                bass_nofuse=True,
                            )
                        )
                    si.on_wait = waits[len(waits) - maxw :]
                    changed = True
                out.append(inst)
            if changed:
                bb.instructions = out


def _mk_nc():
    import concourse.bass as bass

    return bass.Bass(num_devices=NCORES, debug=False, target_bir_lowering=False)


def _load_w(nc, pool, dram_ap, p, f, tag, dtype=None):
    from concourse import mybir

    t = pool.tile([p, f], dtype or mybir.dt.float32, tag=tag)
    nc.sync.dma_start(t[:], dram_ap[:])
    return t


def _proj_cols(nc, tc, ctx, w_t, b_t, srcs, out_slices, act_pool, psum_pool):
    """For each (src columnar tile [64, NPC], dram slice): write
    (w.T @ src + b) in bf16 to the dram slice, chunked by 500 cols."""
    from concourse import mybir

    f32 = mybir.dt.float32
    bf = mybir.dt.bfloat16
    Ident = mybir.ActivationFunctionType.Identity
    NCH = 500
    for (src, dst) in zip(srcs, out_slices):
        for j0 in range(0, NPC, NCH):
            w = min(NCH, NPC - j0)
            ps = psum_pool.tile([HID, NCH], f32, tag="proj")
            nc.tensor.matmul(
                out=ps[:, :w], lhsT=w_t[:], rhs=src[:, j0 : j0 + w],
                start=True, stop=True,
            )
            sb = act_pool.tile([HID, NCH], bf, tag="projsb")
            nc.scalar.activation(sb[:, :w], ps[:, :w], Ident, bias=b_t[:])
            nc.sync.dma_start(dst[:, j0 : j0 + w], sb[:, :w])


def _proj_cols_f32(nc, tc, ctx, w_t, b_t, src, dst, act_pool, psum_pool):
    """Single projection written as f32 (for q tables that the next launch
    transposes on device)."""
    from concourse import mybir

    f32 = mybir.dt.float32
    Ident = mybir.ActivationFunctionType.Identity
    NCH = 500
    for j0 in range(0, NPC, NCH):
        w = min(NCH, NPC - j0)
        ps = psum_pool.tile([HID, NCH], f32, tag="projq")
        nc.tensor.matmul(
            out=ps[:, :w], lhsT=w_t[:], rhs=src[:, j0 : j0 + w],
            start=True, stop=True,
        )
        sb = act_pool.tile([HID, NCH], f32, tag="projqsb")
        nc.scalar.activation(sb[:, :w], ps[:, :w], Ident, bias=b_t[:])
        nc.sync.dma_start(dst[:, j0 : j0 + w], sb[:, :w])


def _consts(nc, tc, ctx):
    from concourse import mybir
    from concourse.masks import make_identity

    cpool = ctx.enter_context(tc.tile_pool(name="const", bufs=1))
    ident_f = cpool.tile([128, 128], mybir.dt.float32, tag="idf")
    make_identity(nc, ident_f[:])
    ident_b = cpool.tile([128, 128], mybir.dt.bfloat16, tag="idb")
    nc.vector.tensor_copy(ident_b[:], ident_f[:])
    return ident_f, ident_b


def _qqt_from_cols(nc, tc, ctx, qT_d):
    """Load q column-table [64, NPC] f32 -> qqt [128, NSB, 128] bf16
    (rows [q; q] per superblock, zero-padded past NPC)."""
    from concourse import mybir

    f32 = mybir.dt.float32
    bf = mybir.dt.bfloat16
    Ident = mybir.ActivationFunctionType.Identity
    qpool = ctx.enter_context(tc.tile_pool(name="q", bufs=1))
    qT = qpool.tile([HID, NPC], f32, tag="qT")
    nc.sync.dma_start(qT[:], qT_d[:])
    qqt = qpool.tile([128, NSB, 128], bf, tag="qqt")
    nc.vector.memset(qqt[:], 0.0)
    for b in range(NSB):
        j0 = b * SBT
        w = min(SBT, NPC - j0)
        nc.scalar.activation(qqt[0:64, b, :w], qT[:, j0 : j0 + w], Ident)
        nc.scalar.activation(qqt[64:128, b, :w], qT[:, j0 : j0 + w], Ident)
    return qqt


def _qrows_from_cols(nc, tc, ctx, qT_d, ident_f):
    """Load q column-table [64, NPC] f32, transpose per superblock into
    qrows [128, NSB, 64] bf16 (row p = q of slot p; pad slots zeroed)."""
    from concourse import mybir

    f32 = mybir.dt.float32
    bf = mybir.dt.bfloat16
    qpool = ctx.enter_context(tc.tile_pool(name="q", bufs=1))
    qT = qpool.tile([HID, NPC], f32, tag="qT")
    nc.sync.dma_start(qT[:], qT_d[:])
    qrows = qpool.tile([128, NSB, HID], bf, tag="qrows")
    nc.vector.memset(qrows[:], 0.0)
    with tc.tile_pool(name="pqt", bufs=2, space="PSUM") as pst:
        for b in range(NSB):
            j0 = b * SBT
            w = min(SBT, NPC - j0)
            ps = pst.tile([128, HID], f32, tag="qtp")
            nc.tensor.transpose(
                out=ps[:w], in_=qT[:, j0 : j0 + w], identity=ident_f[:HID, :HID]
            )
            nc.scalar.copy(qrows[:w, b], ps[:w])
    return qrows


# ---------------------------------------------------------------- edge phase
def _edge_loop(nc, tc, ctx, chunks, streams, ident_b, compute_msg, out_cb,
               stageA=None, stageB=None):
    """Stream per-sb chunks; segment-sum via identity matmul (psT[64, 128] =
    sum_tiles msg.T). streams: list of (dram_ap, np, tag).

    Either compute_msg(tiles, b, t0, gw) -> msg, or a software-pipelined pair
    stageA(tiles, b, t0, gw) -> ctxobj (score side, ends on an ACT op) and
    stageB(ctxobj) -> msg: stageA of chunk i+1 is emitted before stageB of
    chunk i so the DVE works while ACT produces chunk i's activation."""
    from concourse import mybir

    f32 = mybir.dt.float32
    bf = mybir.dt.bfloat16
    ed_pool = ctx.enter_context(tc.tile_pool(name="ed", bufs=2))
    psum_seg = ctx.enter_context(tc.tile_pool(name="pseg", bufs=2, space="PSUM"))

    state = {"psT": None}

    def finish(b, t0, gw, first, last, aobj):
        msg = stageB(aobj) if stageB else aobj
        if first:
            psT_new = psum_seg.tile([HID, 128], f32, tag="psT")
            state["psT"] = psT_new
        psT = state["psT"]
        for gi in range(gw):
            nc.tensor.matmul(
                out=psT[:],
                lhsT=msg[:, gi],
                rhs=ident_b[:],
                start=(first and gi == 0),
                stop=(last and gi == gw - 1),
            )
        if last:
            out_cb(b, psT)

    pend = None
    for (b, t0, gw, first, last) in chunks:
        tiles = []
        for (ap, np_, tag) in streams:
            t = ed_pool.tile([np_, MAXG, ap.shape[2]], bf, tag=tag)
            nc.sync.dma_start(t[:, :gw], ap[:, t0 : t0 + gw, :])
            tiles.append(t)
        aobj = stageA(tiles, b, t0, gw) if stageA else compute_msg(tiles, b, t0, gw)
        if pend is not None:
            finish(*pend)
        pend = (b, t0, gw, first, last, aobj)
    if pend is not None:
        finish(*pend)


def _nrm_tiles(nc, tc, ctx, tt, nrm_d=None, nrmb_d=None):
    from concourse import mybir

    meta_pool = ctx.enter_context(tc.tile_pool(name="meta", bufs=1))
    nrm_t = None
    if nrm_d is not None:
        nrm_t = meta_pool.tile([128, tt], mybir.dt.float32, tag="nrmf")
        nc.sync.dma_start(nrm_t[:], nrm_d[:])
    nrmb_t = None
    if nrmb_d is not None:
        nrmb_t = meta_pool.tile([128, tt], mybir.dt.bfloat16, tag="nrmb")
        nc.sync.dma_start(nrmb_t[:], nrmb_d[:])
    return nrm_t, nrmb_t


# ---------------------------------------------------------------- launch A
def _build_launch_A():
    import concourse.tile as tile
    from concourse import mybir
    from contextlib import ExitStack

    f32 = mybir.dt.float32
    bf = mybir.dt.bfloat16
    nc = _mk_nc()
    xT = nc.dram_tensor("xT", [IN_C, NPC], bf, kind="ExternalInput").ap()
    w1 = nc.dram_tensor("w1", [IN_C, HID], bf, kind="ExternalInput").ap()
    b1 = nc.dram_tensor("b1", [HID, 1], f32, kind="ExternalInput").ap()
    hT_out = nc.dram_tensor("hT_out", [HID, NPC], bf, kind="ExternalOutput").ap()

    with tile.TileContext(nc) as tc, ExitStack() as ctx:
        wpool = ctx.enter_context(tc.tile_pool(name="w", bufs=1))
        xpool = ctx.enter_context(tc.tile_pool(name="x", bufs=1))
        hpool = ctx.enter_context(tc.tile_pool(name="h", bufs=1))
        psum_pool = ctx.enter_context(tc.tile_pool(name="ps", bufs=2, space="PSUM"))

        w1a = _load_w(nc, wpool, w1[0:128, :], 128, HID, "w1a", bf)
        w1b = _load_w(nc, wpool, w1[128:256, :], 128, HID, "w1b", bf)
        b1t = _load_w(nc, wpool, b1, HID, 1, "b1t")
        xa = xpool.tile([128, NPC], bf, tag="xa")
        xb = xpool.tile([128, NPC], bf, tag="xb")
        nc.sync.dma_start(xa[:], xT[0:128, :])
        nc.sync.dma_start(xb[:], xT[128:256, :])

        hT = hpool.tile([HID, NPC], bf)
        NCH = 500
        Relu = mybir.ActivationFunctionType.Relu
        for j0 in range(0, NPC, NCH):
            w = min(NCH, NPC - j0)
            ps = psum_pool.tile([HID, NCH], f32, tag="p1")
            nc.tensor.matmul(out=ps[:, :w], lhsT=w1a[:], rhs=xa[:, j0 : j0 + w], start=True, stop=False)
            nc.tensor.matmul(out=ps[:, :w], lhsT=w1b[:], rhs=xb[:, j0 : j0 + w], start=False, stop=True)
            nc.scalar.activation(hT[:, j0 : j0 + w], ps[:, :w], Relu, bias=b1t[:])
        nc.sync.dma_start(hT_out[:], hT[:])
    _split_multi_waits(nc)
    return nc


# ---------------------------------------------------------------- launch B (layer 1)
def _build_launch_B(tt, chunks):
    import concourse.tile as tile
    from concourse import mybir
    from contextlib import ExitStack

    f32 = mybir.dt.float32
    bf = mybir.dt.bfloat16
    AT = mybir.AluOpType
    Relu = mybir.ActivationFunctionType.Relu
    nc = _mk_nc()

    ed_d = nc.dram_tensor("ed", [128, tt, HID], bf, kind="ExternalInput").ap()
    nrm8b_d = nc.dram_tensor("nrm8b", [128, tt, 8], bf, kind="ExternalInput").ap()
    hT_d = nc.dram_tensor("hT", [HID, NPC], bf, kind="ExternalInput").ap()
    s8_d = nc.dram_tensor("s8", [8, NPC], bf, kind="ExternalInput").ap()
    bv08_d = nc.dram_tensor("bv08", [8, HID], bf, kind="ExternalInput").ap()
    wv0_d = nc.dram_tensor("wv0", [HID, HID], bf, kind="ExternalInput").ap()
    wk2_d = nc.dram_tensor("wk2", [HID, HID], bf, kind="ExternalInput").ap()
    wv2_d = nc.dram_tensor("wv2", [HID, HID], bf, kind="ExternalInput").ap()
    wq2_d = nc.dram_tensor("wq2", [HID, HID], bf, kind="ExternalInput").ap()
    bk2_d = nc.dram_tensor("bk2", [HID, 1], f32, kind="ExternalInput").ap()
    bv2_d = nc.dram_tensor("bv2", [HID, 1], f32, kind="ExternalInput").ap()
    bq2_d = nc.dram_tensor("bq2", [HID, 1], f32, kind="ExternalInput").ap()
    outT_d = nc.dram_tensor("outT", [HID, NPC], bf, kind="ExternalOutput").ap()
    cols_d = nc.dram_tensor("cols", [4 * HID, NPC], bf, kind="ExternalOutput").ap()
    q2T_d = nc.dram_tensor("q2T", [HID, NPC], f32, kind="ExternalOutput").ap()

    with tile.TileContext(nc) as tc, ExitStack() as ctx:
        ident_f, ident_b = _consts(nc, tc, ctx)
        meta_pool = ctx.enter_context(tc.tile_pool(name="meta", bufs=1))
        nrm8b_t = meta_pool.tile([128, tt, 8], bf, tag="nrm8b")
        nc.sync.dma_start(nrm8b_t[:], nrm8b_d[:])
        wpool = ctx.enter_context(tc.tile_pool(name="w", bufs=1))
        hpool = ctx.enter_context(tc.tile_pool(name="h", bufs=1))
        msg_pool = ctx.enter_context(tc.tile_pool(name="msg", bufs=2))
        act_pool = ctx.enter_context(tc.tile_pool(name="act", bufs=2))
        psum_o = ctx.enter_context(tc.tile_pool(name="po", bufs=2, space="PSUM"))
        psum_m = ctx.enter_context(tc.tile_pool(name="pm", bufs=2, space="PSUM"))

        wv0t = _load_w(nc, wpool, wv0_d, HID, HID, "wv0t", bf)
        wk2t = _load_w(nc, wpool, wk2_d, HID, HID, "wk2t", bf)
        wv2t = _load_w(nc, wpool, wv2_d, HID, HID, "wv2t", bf)
        wq2t = _load_w(nc, wpool, wq2_d, HID, HID, "wq2t", bf)
        bk2t = _load_w(nc, wpool, bk2_d, HID, 1, "bk2t")
        bv2t = _load_w(nc, wpool, bv2_d, HID, 1, "bv2t")
        bq2t = _load_w(nc, wpool, bq2_d, HID, 1, "bq2t")
        bv08t = _load_w(nc, wpool, bv08_d, 8, HID, "bv08t", bf)
        s8t = _load_w(nc, wpool, s8_d, 8, NPC, "s8t", bf)
        hT = hpool.tile([HID, NPC], bf, tag="hT")
        nc.sync.dma_start(hT[:], hT_d[:])
        outT = hpool.tile([HID, NPC], bf, tag="outT")

        def compute_msg(tiles, b, t0, gw):
            (ed_t,) = tiles
            msg = msg_pool.tile([128, MAXG, HID], bf, tag="msg")
            nc.vector.tensor_tensor(
                out=msg[:, :gw].rearrange("p c (a h) -> p c a h", h=8),
                in0=ed_t[:, :gw].rearrange("p c (a h) -> p c a h", h=8),
                in1=nrm8b_t[:, t0 : t0 + gw, None, :].to_broadcast([128, gw, 8, 8]),
                op=AT.mult,
            )
            return msg

        def out_cb(b, psT):
            j0 = b * SBT
            w = min(SBT, NPC - j0)
            ST = act_pool.tile([HID, 128], bf, tag="ST")
            nc.scalar.copy(ST[:, :w], psT[:, :w])
            ps2 = psum_o.tile([HID, 128], f32, tag="ps2")
            nc.tensor.matmul(out=ps2[:, :w], lhsT=wv0t[:], rhs=ST[:, :w], start=True, stop=False)
            nc.tensor.matmul(out=ps2[:, :w], lhsT=bv08t[:], rhs=s8t[:, j0 : j0 + w], start=False, stop=True)
            nc.scalar.activation(outT[:, j0 : j0 + w], ps2[:, :w], Relu)

        # hT-sourced projections are independent of the edge loop; issue
        # them first so PE/ACT work on them while edge DMA ramps
        _proj_cols(nc, tc, ctx, wk2t, bk2t, [hT], [cols_d[0:64, :]], act_pool, psum_m)
        _proj_cols(nc, tc, ctx, wv2t, bv2t, [hT], [cols_d[128:192, :]], act_pool, psum_m)

        _edge_loop(nc, tc, ctx, chunks, [(ed_d, 128, "ed")], ident_b, compute_msg, out_cb)

        _proj_cols(nc, tc, ctx, wk2t, bk2t, [outT], [cols_d[64:128, :]], act_pool, psum_m)
        _proj_cols(nc, tc, ctx, wv2t, bv2t, [outT], [cols_d[192:256, :]], act_pool, psum_m)
        _proj_cols_f32(nc, tc, ctx, wq2t, bq2t, outT, q2T_d, act_pool, psum_m)
        nc.sync.dma_start(outT_d[:], outT[:])
    _split_multi_waits(nc)
    return nc


# ---------------------------------------------------------------- launch C (layer 2)
def _build_launch_C(tt, chunks):
    import concourse.tile as tile
    from concourse import mybir
    from contextlib import ExitStack

    f32 = mybir.dt.float32
    bf = mybir.dt.bfloat16
    AT = mybir.AluOpType
    Relu = mybir.ActivationFunctionType.Relu
    Sig = mybir.ActivationFunctionType.Sigmoid
    nc = _mk_nc()
    roww = 4 * HID  # 256: [k0 k1 | v0 v1(d-major)]

    ed_d = nc.dram_tensor("ed", [128, tt, roww], bf, kind="ExternalInput").ap()
    nrm8b_d = nc.dram_tensor("nrm8b", [128, tt, 8], bf, kind="ExternalInput").ap()
    qT_d = nc.dram_tensor("qT", [HID, NPC], f32, kind="ExternalInput").ap()
    hT_d = nc.dram_tensor("hT", [HID, NPC], bf, kind="ExternalInput").ap()
    o1T_d = nc.dram_tensor("o1T", [HID, NPC], bf, kind="ExternalInput").ap()
    wk3a_d = nc.dram_tensor("wk3a", [HID, HID], bf, kind="ExternalInput").ap()
    wk3c_d = nc.dram_tensor("wk3c", [HID, HID], bf, kind="ExternalInput").ap()
    wv3a_d = nc.dram_tensor("wv3a", [HID, HID], bf, kind="ExternalInput").ap()
    wv3c_d = nc.dram_tensor("wv3c", [HID, HID], bf, kind="ExternalInput").ap()
    wq3_d = nc.dram_tensor("wq3", [HID, HID], bf, kind="ExternalInput").ap()
    bk3_d = nc.dram_tensor("bk3", [HID, 1], f32, kind="ExternalInput").ap()
    bv3_d = nc.dram_tensor("bv3", [HID, 1], f32, kind="ExternalInput").ap()
    bq3_d = nc.dram_tensor("bq3", [HID, 1], f32, kind="ExternalInput").ap()
    cols_d = nc.dram_tensor("cols", [6 * HID, NPC], bf, kind="ExternalOutput").ap()
    q3T_d = nc.dram_tensor("q3T", [HID, NPC], f32, kind="ExternalOutput").ap()

    with tile.TileContext(nc) as tc, ExitStack() as ctx:
        ident_f, ident_b = _consts(nc, tc, ctx)
        meta_pool = ctx.enter_context(tc.tile_pool(name="meta", bufs=1))
        nrm8b_t = meta_pool.tile([128, tt, 8], bf, tag="nrm8b")
        nc.sync.dma_start(nrm8b_t[:], nrm8b_d[:])
        qrows = _qrows_from_cols(nc, tc, ctx, qT_d, ident_f)
        wpool = ctx.enter_context(tc.tile_pool(name="w", bufs=1))
        hpool = ctx.enter_context(tc.tile_pool(name="h", bufs=1))
        dk_pool = ctx.enter_context(tc.tile_pool(name="dk", bufs=2))
        sc_pool = ctx.enter_context(tc.tile_pool(name="sc", bufs=2))
        msg_pool = ctx.enter_context(tc.tile_pool(name="msg", bufs=2))
        act_pool = ctx.enter_context(tc.tile_pool(name="act", bufs=2))
        psum_m = ctx.enter_context(tc.tile_pool(name="pm", bufs=2, space="PSUM"))

        wk3at = _load_w(nc, wpool, wk3a_d, HID, HID, "wk3at", bf)
        wk3ct = _load_w(nc, wpool, wk3c_d, HID, HID, "wk3ct", bf)
        wv3at = _load_w(nc, wpool, wv3a_d, HID, HID, "wv3at", bf)
        wv3ct = _load_w(nc, wpool, wv3c_d, HID, HID, "wv3ct", bf)
        wq3t = _load_w(nc, wpool, wq3_d, HID, HID, "wq3t", bf)
        bk3t = _load_w(nc, wpool, bk3_d, HID, 1, "bk3t")
        bv3t = _load_w(nc, wpool, bv3_d, HID, 1, "bv3t")
        bq3t = _load_w(nc, wpool, bq3_d, HID, 1, "bq3t")
        hT = hpool.tile([HID, NPC], bf, tag="hT")
        nc.sync.dma_start(hT[:], hT_d[:])
        o1T = hpool.tile([HID, NPC], bf, tag="o1T")
        nc.sync.dma_start(o1T[:], o1T_d[:])
        o2T = hpool.tile([HID, NPC], bf, tag="o2T")

        def stageA(tiles, b, t0, gw):
            (ed_t,) = tiles
            ke = ed_t[:, :gw, 0 : 2 * HID].rearrange("p c (t d) -> p c t d", t=2)
            dk = dk_pool.tile([128, MAXG, 2, HID], bf, tag="dk")
            nc.vector.tensor_tensor(
                out=dk[:, :gw], in0=ke,
                in1=qrows[:, b : b + 1, None, :].to_broadcast([128, gw, 2, HID]),
                op=AT.mult,
            )
            dk5 = dk[:, :gw].rearrange("p c t (h d) -> p c t h d", h=8)
            r4 = sc_pool.tile([128, MAXG, 2, 8, 4], bf, tag="r4")
            nc.vector.tensor_tensor(
                out=r4[:, :gw], in0=dk5[:, :, :, :, 0:4], in1=dk5[:, :, :, :, 4:8], op=AT.add
            )
            r2 = sc_pool.tile([128, MAXG, 2, 8, 2], bf, tag="r2")
            nc.vector.tensor_tensor(
                out=r2[:, :gw], in0=r4[:, :gw, :, :, 0:2], in1=r4[:, :gw, :, :, 2:4], op=AT.add
            )
            sc = sc_pool.tile([128, MAXG, 2, 8], f32, tag="sc")
            nc.vector.tensor_tensor(
                out=sc[:, :gw], in0=r2[:, :gw, :, :, 0], in1=r2[:, :gw, :, :, 1], op=AT.add
            )
            z = sc_pool.tile([128, MAXG, 8], f32, tag="z")
            nc.vector.tensor_tensor(out=z[:, :gw], in0=sc[:, :gw, 0], in1=sc[:, :gw, 1], op=AT.subtract)
            a0 = sc_pool.tile([128, MAXG, 8], bf, tag="a0")
            nc.scalar.activation(a0[:, :gw], z[:, :gw], Sig)
            return (ed_t, a0, t0, gw)

        def stageB(aobj):
            (ed_t, a0, t0, gw) = aobj
            an0 = sc_pool.tile([128, MAXG, 8], bf, tag="an0")
            nc.vector.tensor_tensor(
                out=an0[:, :gw], in0=a0[:, :gw],
                in1=nrm8b_t[:, t0 : t0 + gw], op=AT.mult,
            )
            an1 = sc_pool.tile([128, MAXG, 8], bf, tag="an1")
            nc.vector.tensor_tensor(
                out=an1[:, :gw], in0=nrm8b_t[:, t0 : t0 + gw], in1=an0[:, :gw],
                op=AT.subtract,
            )
            ve = ed_t[:, :gw, 2 * HID : 4 * HID].rearrange(
                "p c (t d h) -> p c t d h", t=2, d=8
            )
            wv0_ = msg_pool.tile([128, MAXG, 8, 8], bf, tag="wv0")
            nc.vector.tensor_tensor(
                out=wv0_[:, :gw], in0=ve[:, :, 0],
                in1=an0[:, :gw, None, :].to_broadcast([128, gw, 8, 8]),
                op=AT.mult,
            )
            wv1_ = msg_pool.tile([128, MAXG, 8, 8], bf, tag="wv1")
            nc.vector.tensor_tensor(
                out=wv1_[:, :gw], in0=ve[:, :, 1],
                in1=an1[:, :gw, None, :].to_broadcast([128, gw, 8, 8]),
                op=AT.mult,
            )
            msg = msg_pool.tile([128, MAXG, HID], bf, tag="msg")
            nc.vector.tensor_tensor(
                out=msg[:, :gw].rearrange("p c (d h) -> p c d h", d=8),
                in0=wv0_[:, :gw], in1=wv1_[:, :gw], op=AT.add,
            )
            return msg

        def out_cb(b, psT):
            j0 = b * SBT
            w = min(SBT, NPC - j0)
            nc.scalar.activation(o2T[:, j0 : j0 + w], psT[:, :w], Relu)

        _proj_cols(
            nc, tc, ctx, wk3at, bk3t, [hT, o1T],
            [cols_d[0:64, :], cols_d[64:128, :]], act_pool, psum_m,
        )
        _proj_cols(
            nc, tc, ctx, wv3at, bv3t, [hT, o1T],
            [cols_d[192:256, :], cols_d[256:320, :]], act_pool, psum_m,
        )

        _edge_loop(
            nc, tc, ctx, chunks, [(ed_d, 128, "ed")], ident_b, None, out_cb,
            stageA=stageA, stageB=stageB,
        )

        _proj_cols(
            nc, tc, ctx, wk3ct, bk3t, [o2T], [cols_d[128:192, :]], act_pool, psum_m,
        )
        _proj_cols(
            nc, tc, ctx, wv3ct, bv3t, [o2T], [cols_d[320:384, :]], act_pool, psum_m,
        )
        _proj_cols_f32(nc, tc, ctx, wq3t, bq3t, o2T, q3T_d, act_pool, psum_m)
    _split_multi_waits(nc)
    return nc


# ---------------------------------------------------------------- launch D (layer 3 + head)
def _build_launch_D(tt, chunks):
    import concourse.tile as tile
    from concourse import mybir
    from contextlib import ExitStack

    f32 = mybir.dt.float32
    bf = mybir.dt.bfloat16
    AT = mybir.AluOpType
    Relu = mybir.ActivationFunctionType.Relu
    Exp = mybir.ActivationFunctionType.Exp
    Ln = mybir.ActivationFunctionType.Ln
    nc = _mk_nc()
    roww = 6 * HID  # 384: [k0 k1 k2 | v0 v1 v2(d-major)]

    ed_d = nc.dram_tensor("ed", [128, tt, roww], bf, kind="ExternalInput").ap()
    nrm8b_d = nc.dram_tensor("nrm8b", [128, tt, 8], bf, kind="ExternalInput").ap()
    qT_d = nc.dram_tensor("qT", [HID, NPC], f32, kind="ExternalInput").ap()
    w2_d = nc.dram_tensor("w2", [HID, OUT_C], bf, kind="ExternalInput").ap()
    b2bc_d = nc.dram_tensor("b2bc", [128, OUT_C], f32, kind="ExternalInput").ap()
    y_d = nc.dram_tensor("y", [NPC, OUT_C], f32, kind="ExternalOutput").ap()

    with tile.TileContext(nc) as tc, ExitStack() as ctx:
        ident_f, ident_b = _consts(nc, tc, ctx)
        meta_pool = ctx.enter_context(tc.tile_pool(name="meta", bufs=1))
        nrm8b_t = meta_pool.tile([128, tt, 8], bf, tag="nrm8b")
        nc.sync.dma_start(nrm8b_t[:], nrm8b_d[:])
        qrows = _qrows_from_cols(nc, tc, ctx, qT_d, ident_f)
        wpool = ctx.enter_context(tc.tile_pool(name="w", bufs=1))
        dk_pool = ctx.enter_context(tc.tile_pool(name="dk", bufs=2))
        sc_pool = ctx.enter_context(tc.tile_pool(name="sc", bufs=2))
        msg_pool = ctx.enter_context(tc.tile_pool(name="msg", bufs=2))
        act_pool = ctx.enter_context(tc.tile_pool(name="act", bufs=2))
        sm_pool = ctx.enter_context(tc.tile_pool(name="sm", bufs=2))
        psum_lg = ctx.enter_context(tc.tile_pool(name="plg", bufs=2, space="PSUM"))

        w2t = _load_w(nc, wpool, w2_d, HID, OUT_C, "w2t", bf)
        b2t = _load_w(nc, wpool, b2bc_d, 128, OUT_C, "b2t")

        def stageA(tiles, b, t0, gw):
            (ed_t,) = tiles
            ke = ed_t[:, :gw, 0 : 3 * HID].rearrange("p c (t d) -> p c t d", t=3)
            dk = dk_pool.tile([128, MAXG, 3, HID], bf, tag="dk")
            nc.vector.tensor_tensor(
                out=dk[:, :gw], in0=ke,
                in1=qrows[:, b : b + 1, None, :].to_broadcast([128, gw, 3, HID]),
                op=AT.mult,
            )
            dk5 = dk[:, :gw].rearrange("p c t (h d) -> p c t h d", h=8)
            r4 = sc_pool.tile([128, MAXG, 3, 8, 4], bf, tag="r4")
            nc.vector.tensor_tensor(
                out=r4[:, :gw], in0=dk5[:, :, :, :, 0:4], in1=dk5[:, :, :, :, 4:8], op=AT.add
            )
            r2 = sc_pool.tile([128, MAXG, 3, 8, 2], bf, tag="r2")
            nc.vector.tensor_tensor(
                out=r2[:, :gw], in0=r4[:, :gw, :, :, 0:2], in1=r4[:, :gw, :, :, 2:4], op=AT.add
            )
            sc = sc_pool.tile([128, MAXG, 3, 8], f32, tag="sc")
            nc.vector.tensor_tensor(
                out=sc[:, :gw], in0=r2[:, :gw, :, :, 0], in1=r2[:, :gw, :, :, 1], op=AT.add
            )
            ee = sc_pool.tile([128, MAXG, 3, 8], bf, tag="ee")
            nc.scalar.activation(ee[:, :gw], sc[:, :gw], Exp)
            return (ed_t, ee, t0, gw)

        def stageB(aobj):
            (ed_t, ee, t0, gw) = aobj
            dd1 = sc_pool.tile([128, MAXG, 8], bf, tag="dd1")
            nc.vector.tensor_tensor(out=dd1[:, :gw], in0=ee[:, :gw, 0], in1=ee[:, :gw, 1], op=AT.add)
            dd = sc_pool.tile([128, MAXG, 8], f32, tag="dd")
            nc.vector.tensor_tensor(out=dd[:, :gw], in0=dd1[:, :gw], in1=ee[:, :gw, 2], op=AT.add)
            rr = sc_pool.tile([128, MAXG, 8], bf, tag="rr")
            with nc.allow_low_precision(reason="attn denom fine in bf16"):
                nc.vector.reciprocal(rr[:, :gw], dd[:, :gw])
            rn = sc_pool.tile([128, MAXG, 8], bf, tag="rn")
            nc.vector.tensor_tensor(
                out=rn[:, :gw], in0=rr[:, :gw],
                in1=nrm8b_t[:, t0 : t0 + gw], op=AT.mult,
            )
            aa = sc_pool.tile([128, MAXG, 3, 8], bf, tag="aa")
            nc.vector.tensor_tensor(
                out=aa[:, :gw], in0=ee[:, :gw],
                in1=rn[:, :gw, None, :].to_broadcast([128, gw, 3, 8]),
                op=AT.mult,
            )
            ve = ed_t[:, :gw, 3 * HID : 6 * HID].rearrange(
                "p c (t d h) -> p c t d h", t=3, d=8
            )
            wv_ = msg_pool.tile([128, MAXG, 3, 8, 8], bf, tag="wv")
            nc.vector.tensor_tensor(
                out=wv_[:, :gw], in0=ve,
                in1=aa[:, :gw, :, None, :].to_broadcast([128, gw, 3, 8, 8]),
                op=AT.mult,
            )
            msg1 = msg_pool.tile([128, MAXG, 8, 8], bf, tag="msg1")
            nc.vector.tensor_tensor(out=msg1[:, :gw], in0=wv_[:, :gw, 0], in1=wv_[:, :gw, 1], op=AT.add)
            msg = msg_pool.tile([128, MAXG, HID], bf, tag="msg")
            nc.vector.tensor_tensor(
                out=msg[:, :gw].rearrange("p c (d h) -> p c d h", d=8),
                in0=msg1[:, :gw], in1=wv_[:, :gw, 2], op=AT.add,
            )
            return msg

        def out_cb(b, psT):
            j0 = b * SBT
            w = min(SBT, NPC - j0)
            o3T = act_pool.tile([HID, 128], bf, tag="o3T")
            nc.scalar.activation(o3T[:, :w], psT[:, :w], Relu)
            lg = psum_lg.tile([128, OUT_C], f32, tag="lg")
            nc.tensor.matmul(out=lg[:w], lhsT=o3T[:, :w], rhs=w2t[:], start=True, stop=True)
            logits = sm_pool.tile([128, OUT_C], f32, tag="logits")
            nc.vector.tensor_tensor(out=logits[:w], in0=lg[:w], in1=b2t[:w], op=AT.add)
            nlmax = sm_pool.tile([128, 1], f32, tag="nlmax")
            nc.vector.tensor_reduce(
                out=nlmax[:w], in_=logits[:w], axis=mybir.AxisListType.X,
                op=AT.max, negate=True,
            )
            eb = sm_pool.tile([128, OUT_C], f32, tag="eb")
            esum = sm_pool.tile([128, 1], f32, tag="esum")
            nc.scalar.activation(
                eb[:w], logits[:w], Exp, bias=nlmax[:w], accum_out=esum[:w]
            )
            lse = sm_pool.tile([128, 1], f32, tag="lse")
            nc.scalar.activation(lse[:w], esum[:w], Ln)
            off = sm_pool.tile([128, 1], f32, tag="off")
            nc.vector.tensor_tensor(out=off[:w], in0=lse[:w], in1=nlmax[:w], op=AT.subtract)
            yy = sm_pool.tile([128, OUT_C], f32, tag="yy")
            nc.vector.tensor_tensor(
                out=yy[:w], in0=logits[:w],
                in1=off[:w].to_broadcast([w, OUT_C]), op=AT.subtract,
            )
            nc.sync.dma_start(y_d[j0 : j0 + w, :], yy[:w])

        _edge_loop(
            nc, tc, ctx, chunks, [(ed_d, 128, "ed")], ident_b, None, out_cb,
            stageA=stageA, stageB=stageB,
        )
    _split_multi_waits(nc)
    return nc


# ---------------------------------------------------------------- host gather
def _u16(a):
    return a.view(np.uint16)


def _gather_rows(tab, eidx):
    """[128, TT, W] bf16 rows gathered by global src id."""
    return _u16(tab)[eidx].view(BF16)


def _gather_ed(ktab, vtab, eidx):
    """[128, TT, kw+vw] bf16: k rows then v rows (both by global src id)."""
    tt = eidx.shape[1]
    kw = ktab.shape[1]
    vw = vtab.shape[1]
    out = np.empty((128, tt, kw + vw), dtype=np.uint16)
    out[:, :, :kw] = _u16(ktab)[eidx]
    out[:, :, kw:] = _u16(vtab)[eidx]
    return out.view(BF16)


def _nrm8(meta):
    n8 = np.ascontiguousarray(
        np.broadcast_to(meta["nrm"][:, :, None], meta["nrm"].shape + (8,))
    )
    return n8, n8.astype(BF16)


def _scatter_tab(cols_list, ids, lo, hi, dtype=BF16):
    """tab[global_id] = cols[lo:hi].T for each core."""
    tab = np.empty((N, hi - lo), dtype=dtype)
    for c in range(NCORES):
        tab[ids[c]] = cols_list[c][lo:hi].T
    return tab


# ---------------------------------------------------------------- driver
def kernel(x, edge_index, lin1_w, lin1_b, wq, bq, wk, bk, wv, bv, lin2_w, lin2_b):
    _install_fixups()
    from concourse.bass_utils import run_bass_kernel_spmd

    x = np.asarray(x, dtype=np.float32)
    lin1_w = np.asarray(lin1_w, np.float32)
    lin1_b = np.asarray(lin1_b, np.float32)
    wq = np.asarray(wq, np.float32)
    bq = np.asarray(bq, np.float32)
    wk = np.asarray(wk, np.float32)
    bk = np.asarray(bk, np.float32)
    wv = np.asarray(wv, np.float32)
    bv = np.asarray(bv, np.float32)
    lin2_w = np.asarray(lin2_w, np.float32)
    lin2_b = np.asarray(lin2_b, np.float32)
    isd = np.float32(1.0 / np.sqrt(DH))

    metas, tps, tt, chunks, s_all, ids = _preprocess(np.asarray(edge_index))

    key = ("progs", tps, tt)
    if key not in _CACHE:
        _CACHE[key] = (
            _build_launch_A(),
            _build_launch_B(tt, chunks),
            _build_launch_C(tt, chunks),
            _build_launch_D(tt, chunks),
        )
    ncA, ncB, ncC, ncD = _CACHE[key]
    cores = list(range(NCORES))

    # ---- launch A: h = relu(x @ W1 + b1), columnar bf16
    xT = np.ascontiguousarray(x.T).astype(BF16)
    w1_bf = lin1_w.astype(BF16)
    in_maps = [
        dict(
            xT=np.ascontiguousarray(xT[:, ids[c]]),
            w1=w1_bf,
            b1=lin1_b[:, None],
        )
        for c in cores
    ]
    resA = run_bass_kernel_spmd(ncA, in_maps, cores)
    hT = [np.asarray(resA.results[c]["hT_out"]) for c in cores]
    h_tab = np.empty((N, HID), dtype=BF16)
    for c in cores:
        h_tab[ids[c]] = hT[c].T

    # ---- launch B: layer 1 (attn == identity) + k2/v2/q2 tables
    nrm8b_all = []
    for c in cores:
        _, bb = _nrm8(metas[c])
        nrm8b_all.append(bb)
    s8 = []
    for c in cores:
        a = np.zeros((8, NPC), dtype=BF16)
        a[0] = s_all[ids[c]].astype(BF16)
        s8.append(a)
    bv08 = np.zeros((8, HID), dtype=BF16)
    bv08[0] = bv[0].astype(BF16)
    in_maps = [
        dict(
            ed=_gather_rows(h_tab, metas[c]["eidx"]),
            nrm8b=nrm8b_all[c],
            hT=hT[c],
            s8=s8[c],
            bv08=bv08,
            wv0=wv[0].astype(BF16),
            wk2=wk[1].astype(BF16),
            wv2=wv[1][:, PRM].astype(BF16),
            wq2=(wq[1] * isd).astype(BF16),
            bk2=bk[1][:, None],
            bv2=bv[1][PRM][:, None],
            bq2=(bq[1] * isd)[:, None],
        )
        for c in cores
    ]
    resB = run_bass_kernel_spmd(ncB, in_maps, cores)
    o1T = [np.asarray(resB.results[c]["outT"]) for c in cores]
    colsB = [np.asarray(resB.results[c]["cols"]) for c in cores]
    q2T = [np.asarray(resB.results[c]["q2T"]) for c in cores]
    ktab2 = _scatter_tab(colsB, ids, 0, 128)
    vtab2 = _scatter_tab(colsB, ids, 128, 256)

    # ---- launch C: layer 2 + k3/v3/q3 tables
    in_maps = [
        dict(
            ed=_gather_ed(ktab2, vtab2, metas[c]["eidx"]),
            nrm8b=nrm8b_all[c],
            qT=q2T[c],
            hT=hT[c],
            o1T=o1T[c],
            wk3a=wk[2].astype(BF16),
            wk3c=wk[2][PRM, :].astype(BF16),
            wv3a=wv[2][:, PRM].astype(BF16),
            wv3c=wv[2][PRM, :][:, PRM].astype(BF16),
            wq3=((wq[2] * isd)[PRM, :]).astype(BF16),
            bk3=bk[2][:, None],
            bv3=bv[2][PRM][:, None],
            bq3=(bq[2] * isd)[:, None],
        )
        for c in cores
    ]
    resC = run_bass_kernel_spmd(ncC, in_maps, cores)
    colsC = [np.asarray(resC.results[c]["cols"]) for c in cores]
    q3T = [np.asarray(resC.results[c]["q3T"]) for c in cores]
    ktab3 = _scatter_tab(colsC, ids, 0, 192)
    vtab3 = _scatter_tab(colsC, ids, 192, 384)

    # ---- launch D: layer 3 + classifier head + log_softmax
    b2bc = np.ascontiguousarray(np.broadcast_to(lin2_b[None, :], (128, OUT_C)))
    in_maps = [
        dict(
            ed=_gather_ed(ktab3, vtab3, metas[c]["eidx"]),
            nrm8b=nrm8b_all[c],
            qT=q3T[c],
            w2=lin2_w[PRM, :].astype(BF16),
            b2bc=b2bc,
        )
        for c in cores
    ]
    resD = run_bass_kernel_spmd(ncD, in_maps, cores)
    y = np.empty((N, OUT_C), dtype=np.float32)
    for c in cores:
        y[ids[c]] = np.asarray(resD.results[c]["y"], dtype=np.float32)
    return y
